# revision 13
# baseline (speedup 1.0000x reference)
"""Trainium2 Bass kernel for nn_Correlation (stereo cost volume).

  out[b, d, h, w] = mean_c( x[b,c,h,w] * y[b,c,h,w-d] ),  w >= d else 0
  B=8, C=32, H=256, W=512, D=48  (maxdisp=48)

Sharding: data-parallel over batch B across the 8 NeuronCores (one batch
element per core).  Each core computes its full [D, H, W] cost volume.

Per-core algorithm (bf16 matmul inputs, fp32 psum, fp16 output):
  - x/y rows are loaded with a casting SWDGE DMA (f32 HBM -> bf16 SBUF)
    in two 32-partition slabs (partitions 0-31 and 64-95) so consecutive
    matmuls alternate PE row groups and LDWEIGHTS overlaps the running
    matmul.  bf16 runs the PE ~2x faster than fp32.
  - Per (h, 128-col w-tile): one PE matmul, K=C=32, stationary = X
    columns [32,128], moving = Y window [32,175].  psum[j, u] =
    <x_col(w0+j), y_col(w0+u-47)>, so the 48 outputs of column j sit on
    the diagonal u = j..j+47 (reversed d).
  - Pairs of h-rows share one 2-bank psum tile (g at cols [0:350),
    g+1 at [512:862)); a single DVE/ACT drain scales both by 1/32 and
    writes them into an fp16 G8 tile [128, 5600] that interleaves
    EIGHT h-rows element-wise (col = u*8 + hsub).  Writing h-PAIRS as
    4-byte granules (stride 16 B) costs ~1.3 cyc/col vs ~5 for single
    2-byte strided writes - the drains were the v2 bottleneck.
  - The w<d zone (cols u<47 of w-tile 0) is never drained; a single
    memset zeroes it per tile (disjoint bytes from the drains).
  - Each h-block's G8 tile is dumped contiguously to a DRAM scratch
    block, and a skewed DRAM->DRAM DMA walks the 48d x 8h diagonal
    runs (768 B contiguous per (j,wt)) straight into the output,
    fully contiguous 393 KB per block in [hblk, j, wt, (d_rev,hsub)]
    layout.  The DRAM bounce is forced: SBUF-side per-partition byte
    skew only lowers correctly for a single 4-partition group at
    partition 0 (HW resets the skew every 4 partitions; walrus
    codegen cannot encode byte-carrying partition steps in outer AP
    dims at all), so a direct skewed SBUF->DRAM dump is impossible.
    The 8-h interleave is what makes the gather's segments 768 B
    instead of 96 B - the per-h variant was DMA-packet-bound.
  - The host casts back to fp32 and unpermutes [hblk,j,wt,d_rev,hsub]
    -> [d, h, w].
"""

import sys

sys.path.insert(0, "/opt/trn_rl_repo")

import numpy as np
from contextlib import ExitStack

import concourse.bass as bass
import concourse.tile as tile
from concourse import mybir
from concourse import bass_utils

B = 8
C = 32
H = 256
W = 512
D = 48
NW = W // 128           # 4 w-tiles per row
MMN = 128 + D - 1       # 175 moving columns per matmul
LEAD = D - 1            # 47
HB = 8                  # h rows interleaved per output block
NBLK = H // HB          # 32 h-blocks
GW8 = NW * MMN * HB     # 5600 G8 cols (u * 8 + hsub)
DH = D * HB             # 384 contiguous elements per diagonal run (768 B)
PW = 63 * HB + DH       # 888: dumped band cols per w-tile per row-half
PROW = NW * PW          # 3552: scratch cols per row (one half)
PBLK = 64 * PROW        # 227328: scratch elems per piece
SBLK = 2 * PBLK         # per-block scratch elems


def _split_waits(nc, max_waits=1):
    """Walrus codegen accepts at most ONE sync wait per instruction; Tile
    attaches several.  Split extra waits onto preceding NoOps on the same
    engine queue (dispatch is in-order, waits gate dispatch)."""
    for fn in nc.m.functions:
        for blk in fn.blocks:
            newl = []
            changed = False
            for inst in blk.instructions:
                si = getattr(inst, "sync_info", None)
                ow = list(si.on_wait) if si is not None and si.on_wait else []
                if len(ow) > max_waits and inst.engine is not None:
                    for k, wcond in enumerate(ow[:-max_waits]):
                        newl.append(mybir.InstNoOp(
                            name=f"{inst.name}w{k}",
                            engine=inst.engine,
                            sync_info=mybir.SyncInfo(on_wait=[wcond],
                                                     on_update=[]),
                        ))
                    inst.sync_info = mybir.SyncInfo(
                        on_wait=ow[-max_waits:],
                        on_update=list(si.on_update) if si.on_update else [])
                    changed = True
                newl.append(inst)
            if changed:
                blk.instructions = newl


def _emit_body(ctx, tc, x_ap, y_ap, o_ap, act_frac=0.5):
    nc = tc.nc
    o_t = o_ap.tensor
    yflat = y_ap.rearrange("c h w -> c (h w)")

    # DRAM scratch: two 64-row pieces per h-block.  The gather only reads
    # cols [j*8, j*8+384) per (j, w-tile), so rows 0-63 need band cols
    # [0, 888) and rows 64-127 need [512, 1400) per w-tile - dumping just
    # those pieces cuts the dump from 45.9 MB to 29.1 MB.
    gd = nc.dram_tensor("gd", [NBLK * SBLK], mybir.dt.float16,
                        kind="Internal")

    xpool = ctx.enter_context(tc.tile_pool(name="xp", bufs=2))
    ypool = ctx.enter_context(tc.tile_pool(name="yp", bufs=2))
    gpool = ctx.enter_context(tc.tile_pool(name="gp", bufs=6))
    ppool = ctx.enter_context(tc.tile_pool(name="pp", bufs=3, space="PSUM"))

    inv_c = 1.0 / C
    dcount = 0
    n_iter = H // (2 * HB)          # 16 iterations, 2 h-blocks each

    for it in range(n_iter):
        h0 = it * 2 * HB            # slab A rows [h0, h0+8), B [h0+8, h0+16)
        pcs = (2 * it, 2 * it + 1)
        bases = (0, 64)

        xt = xpool.tile([128, HB * W], mybir.dt.bfloat16,
                        name=f"xt{it}", tag="xt")
        yt = ypool.tile([128, LEAD + HB * W], mybir.dt.bfloat16,
                        name=f"yt{it}", tag="yt")

        # casting loads (SWDGE): f32 HBM -> bf16 SBUF
        nc.gpsimd.dma_start(xt[0:C, :], x_ap[:, h0:h0 + HB, :])
        nc.gpsimd.dma_start(xt[64:64 + C, :], x_ap[:, h0 + HB:h0 + 2 * HB, :])
        if it == 0:
            # no rows before row 0: lead cols stay unloaded; the very first
            # w-tile uses a shrunk moving window instead
            nc.gpsimd.dma_start(yt[0:C, LEAD:], yflat[:, 0:HB * W])
        else:
            nc.gpsimd.dma_start(yt[0:C, :],
                                yflat[:, h0 * W - LEAD:(h0 + HB) * W])
        nc.gpsimd.dma_start(yt[64:64 + C, :],
                            yflat[:, (h0 + HB) * W - LEAD:(h0 + 2 * HB) * W])

        gts = [gpool.tile([128, GW8], mybir.dt.float16,
                          name=f"gt{it}_{s}", tag="gt") for s in range(2)]
        for s in range(2):
            # zero the w<d zone (u<47, all hsub); drains never touch it
            nc.vector.memset(gts[s][:, 0:LEAD * HB], 0.0)

        for q in range(HB // 2):         # h-row pairs g = 2q, 2q+1
            ps = {}
            for s in range(2):
                for half in range(NW // 2):
                    ps[s, half] = ppool.tile(
                        [128, 1024], mybir.dt.float32,
                        name=f"ps{it}_{q}_{s}_{half}", tag="ps",
                        padded_shape=[128, 1024])
            for p in range(2):
                g = 2 * q + p
                for half in range(NW // 2):
                    for wsub in range(2):
                        wt = half * 2 + wsub
                        for s in range(2):
                            base = bases[s]
                            lhs = xt[base:base + C,
                                     g * W + wt * 128: g * W + wt * 128 + 128]
                            lo = LEAD if (it == 0 and g == 0 and s == 0
                                          and wt == 0) else 0
                            rhs = yt[base:base + C,
                                     g * W + wt * 128 + lo:
                                     g * W + wt * 128 + MMN]
                            nc.tensor.matmul(
                                ps[s, half][:, 512 * p + wsub * MMN + lo:
                                            512 * p + (wsub + 1) * MMN],
                                lhs, rhs, start=True, stop=True)

            for s in range(2):
                gt = gts[s]
                for half in range(NW // 2):
                    # half 0 skips u<47 (w-tile 0's w<d zone, memset to 0)
                    lo = LEAD if half == 0 else 0
                    gfull = gt[:, :]
                    dst = bass.AP(
                        gfull.tensor,
                        gfull.offset + (half * 2 * MMN + lo) * HB + 2 * q,
                        [[GW8, 128], [HB, 2 * MMN - lo], [1, 2]])
                    pfull = ps[s, half][:, :]
                    src = bass.AP(
                        pfull.tensor, pfull.offset + lo,
                        [[1024, 128], [1, 2 * MMN - lo], [512, 2]])
                    if (dcount % 13) < act_frac * 13:
                        nc.scalar.mul(dst, src, inv_c)
                    else:
                        nc.vector.tensor_scalar_mul(dst, src, inv_c)
                    dcount += 1

        for s in range(2):
            pc = pcs[s]
            eng = nc.sync if s == 0 else nc.scalar
            gt3d = gts[s][:, :].rearrange("p (w c) -> p w c", c=MMN * HB)
            for half in range(2):        # row-halves j in [0,64), [64,128)
                scr0 = pc * SBLK + half * PBLK
                dmp_src = gt3d[64 * half:64 * half + 64, :,
                               half * 512:half * 512 + PW]
                dmp_dst = bass.AP(gd, scr0, [[PROW, 64], [PW, NW], [1, PW]])
                eng.dma_start(dmp_dst, dmp_src)
                # skewed extraction: 768B diagonal runs -> contiguous output
                g_src = bass.AP(gd, scr0,
                                [[PROW + HB, 64], [PW, NW], [1, DH]])
                g_dst = bass.AP(o_t, pc * (128 * NW * DH)
                                + half * (64 * NW * DH),
                                [[NW * DH, 64], [DH, NW], [1, DH]])
                eng.dma_start(g_dst, g_src)


def _build_kernel():
    nc = bass.Bass(trn_type="TRN2", target_bir_lowering=False)
    x_d = nc.dram_tensor("x", [C, H, W], mybir.dt.float32, kind="ExternalInput")
    y_d = nc.dram_tensor("y", [C, H, W], mybir.dt.float32, kind="ExternalInput")
    o_d = nc.dram_tensor("o", [NBLK, 128, NW, DH], mybir.dt.float16,
                         kind="ExternalOutput")
    with ExitStack() as ctx:
        tc = ctx.enter_context(tile.TileContext(nc))
        _emit_body(ctx, tc, x_d.ap(), y_d.ap(), o_d.ap())
    _split_waits(nc)
    return nc


_NC_CACHE = None


def _get_nc():
    global _NC_CACHE
    if _NC_CACHE is None:
        _NC_CACHE = _build_kernel()
    return _NC_CACHE


def kernel(x: np.ndarray, y: np.ndarray, maxdisp=48) -> np.ndarray:
    assert int(maxdisp) == D
    x = np.ascontiguousarray(np.asarray(x, dtype=np.float32))
    y = np.ascontiguousarray(np.asarray(y, dtype=np.float32))
    assert x.shape == (B, C, H, W) and y.shape == (B, C, H, W)

    nc = _get_nc()
    in_maps = [{"x": x[b], "y": y[b]} for b in range(B)]
    res = bass_utils.run_bass_kernel_spmd(nc, in_maps, core_ids=list(range(B)))

    out = np.empty((B, D, H, W), dtype=np.float32)
    for b in range(B):
        ob = np.asarray(res.results[b]["o"])      # [NBLK, 128, NW, DH] fp16
        arr = ob.reshape(NBLK, 128, NW, D, HB)    # [hblk, j, wt, d_rev, hsub]
        out[b] = (arr[:, :, :, ::-1, :]
                  .transpose(3, 0, 4, 2, 1)       # [d, hblk, hsub, wt, j]
                  .reshape(D, H, W)
                  .astype(np.float32))
    return out


if __name__ == "__main__":
    rng = np.random.default_rng(0)
    x = rng.standard_normal((B, C, H, W), dtype=np.float32)
    y = rng.standard_normal((B, C, H, W), dtype=np.float32)
    out = kernel(x=x, y=y, maxdisp=D)
    print("kernel output:", out.shape, out.dtype)


# revision 16
# speedup vs baseline: 1.0697x; 1.0697x over previous
"""Trainium2 Bass kernel for nn_Correlation (stereo cost volume).

  out[b, d, h, w] = mean_c( x[b,c,h,w] * y[b,c,h,w-d] ),  w >= d else 0
  B=8, C=32, H=256, W=512, D=48  (maxdisp=48)

Sharding: data-parallel over batch B across the 8 NeuronCores (one batch
element per core).  Each core computes its full [D, H, W] cost volume.

Per-core algorithm (bf16 matmul inputs, fp32 psum, fp16 output):
  - x/y rows are loaded with a casting SWDGE DMA (f32 HBM -> bf16 SBUF)
    in two 32-partition slabs (partitions 0-31 and 64-95) so consecutive
    matmuls alternate PE row groups and LDWEIGHTS overlaps the running
    matmul.  bf16 runs the PE ~2x faster than fp32.
  - Per (h, 128-col w-tile): one PE matmul, K=C=32, stationary = X
    columns [32,128], moving = Y window [32,175].  psum[j, u] =
    <x_col(w0+j), y_col(w0+u-47)>, so the 48 outputs of column j sit on
    the diagonal u = j..j+47 (reversed d).
  - Pairs of h-rows share one 2-bank psum tile (g at cols [0:350),
    g+1 at [512:862)); a single DVE/ACT drain scales both by 1/32 and
    writes them into an fp16 G8 tile [128, 5600] that interleaves
    EIGHT h-rows element-wise (col = u*8 + hsub).  Writing h-PAIRS as
    4-byte granules (stride 16 B) costs ~1.3 cyc/col vs ~5 for single
    2-byte strided writes - the drains were the v2 bottleneck.
  - The w<d zone (cols u<47 of w-tile 0) is never drained; a single
    memset zeroes it per tile (disjoint bytes from the drains).
  - Each h-block's G8 tile is dumped contiguously to a DRAM scratch
    block, and a skewed DRAM->DRAM DMA walks the 48d x 8h diagonal
    runs (768 B contiguous per (j,wt)) straight into the output,
    fully contiguous 393 KB per block in [hblk, j, wt, (d_rev,hsub)]
    layout.  The DRAM bounce is forced: SBUF-side per-partition byte
    skew only lowers correctly for a single 4-partition group at
    partition 0 (HW resets the skew every 4 partitions; walrus
    codegen cannot encode byte-carrying partition steps in outer AP
    dims at all), so a direct skewed SBUF->DRAM dump is impossible.
    The 8-h interleave is what makes the gather's segments 768 B
    instead of 96 B - the per-h variant was DMA-packet-bound.
  - The host casts back to fp32 and unpermutes [hblk,j,wt,d_rev,hsub]
    -> [d, h, w].
"""

import sys

sys.path.insert(0, "/opt/trn_rl_repo")

import numpy as np
from contextlib import ExitStack

import concourse.bass as bass
import concourse.tile as tile
from concourse import mybir
from concourse import bass_utils

B = 8
C = 32
H = 256
W = 512
D = 48
NW = W // 128           # 4 w-tiles per row
MMN = 128 + D - 1       # 175 moving columns per matmul
LEAD = D - 1            # 47
HB = 8                  # h rows interleaved per output block
NBLK = H // HB          # 32 h-blocks
GW8 = NW * MMN * HB     # 5600 G8 cols (u * 8 + hsub)
DH = D * HB             # 384 contiguous elements per diagonal run (768 B)
PW = 63 * HB + DH       # 888: dumped band cols per w-tile per row-half
PROW = NW * PW          # 3552: scratch cols per row (one half)
PBLK = 64 * PROW        # 227328: scratch elems per piece
SBLK = 2 * PBLK         # per-block scratch elems


def _split_waits(nc, max_waits=1):
    """Walrus codegen accepts at most ONE sync wait per instruction; Tile
    attaches several.  Split extra waits onto preceding NoOps on the same
    engine queue (dispatch is in-order, waits gate dispatch)."""
    for fn in nc.m.functions:
        for blk in fn.blocks:
            newl = []
            changed = False
            for inst in blk.instructions:
                si = getattr(inst, "sync_info", None)
                ow = list(si.on_wait) if si is not None and si.on_wait else []
                if len(ow) > max_waits and inst.engine is not None:
                    for k, wcond in enumerate(ow[:-max_waits]):
                        newl.append(mybir.InstNoOp(
                            name=f"{inst.name}w{k}",
                            engine=inst.engine,
                            sync_info=mybir.SyncInfo(on_wait=[wcond],
                                                     on_update=[]),
                        ))
                    inst.sync_info = mybir.SyncInfo(
                        on_wait=ow[-max_waits:],
                        on_update=list(si.on_update) if si.on_update else [])
                    changed = True
                newl.append(inst)
            if changed:
                blk.instructions = newl


def _emit_body(ctx, tc, x_ap, y_ap, o_ap, act_frac=0.4):
    nc = tc.nc
    o_t = o_ap.tensor
    yflat = y_ap.rearrange("c h w -> c (h w)")

    # DRAM scratch: two 64-row pieces per h-block.  The gather only reads
    # cols [j*8, j*8+384) per (j, w-tile), so rows 0-63 need band cols
    # [0, 888) and rows 64-127 need [512, 1400) per w-tile - dumping just
    # those pieces cuts the dump from 45.9 MB to 29.1 MB.
    gd = nc.dram_tensor("gd", [NBLK * SBLK], mybir.dt.float16,
                        kind="Internal")

    xpool = ctx.enter_context(tc.tile_pool(name="xp", bufs=2))
    ypool = ctx.enter_context(tc.tile_pool(name="yp", bufs=2))
    gpool = ctx.enter_context(tc.tile_pool(name="gp", bufs=6))
    ppool = ctx.enter_context(tc.tile_pool(name="pp", bufs=3, space="PSUM"))

    inv_c = 1.0 / C
    dcount = 0
    n_iter = H // (2 * HB)          # 16 iterations, 2 h-blocks each
    # Gathers are emitted one iteration AFTER their dump so the gather's
    # semaphore wait (dump completion) is already satisfied at dispatch -
    # otherwise the in-order engine queue stalls ~12us per gather behind
    # the wait (measured: issuing them back-to-back on the scalar ring
    # stalled the drains and cost +53us overall).
    pending = []

    for it in range(n_iter):
        h0 = it * 2 * HB            # slab A rows [h0, h0+8), B [h0+8, h0+16)
        pcs = (2 * it, 2 * it + 1)
        bases = (0, 64)

        xt = xpool.tile([128, HB * W], mybir.dt.bfloat16,
                        name=f"xt{it}", tag="xt")
        yt = ypool.tile([128, LEAD + HB * W], mybir.dt.bfloat16,
                        name=f"yt{it}", tag="yt")

        # casting loads (SWDGE): f32 HBM -> bf16 SBUF
        nc.gpsimd.dma_start(xt[0:C, :], x_ap[:, h0:h0 + HB, :])
        nc.gpsimd.dma_start(xt[64:64 + C, :], x_ap[:, h0 + HB:h0 + 2 * HB, :])
        if it == 0:
            # no rows before row 0: lead cols stay unloaded; the very first
            # w-tile uses a shrunk moving window instead
            nc.gpsimd.dma_start(yt[0:C, LEAD:], yflat[:, 0:HB * W])
        else:
            nc.gpsimd.dma_start(yt[0:C, :],
                                yflat[:, h0 * W - LEAD:(h0 + HB) * W])
        nc.gpsimd.dma_start(yt[64:64 + C, :],
                            yflat[:, (h0 + HB) * W - LEAD:(h0 + 2 * HB) * W])

        gts = [gpool.tile([128, GW8], mybir.dt.float16,
                          name=f"gt{it}_{s}", tag="gt") for s in range(2)]
        for s in range(2):
            # zero the w<d zone (u<47, all hsub); drains never touch it
            nc.vector.memset(gts[s][:, 0:LEAD * HB], 0.0)

        for q in range(HB // 2):         # h-row pairs g = 2q, 2q+1
            ps = {}
            for s in range(2):
                for half in range(NW // 2):
                    ps[s, half] = ppool.tile(
                        [128, 1024], mybir.dt.float32,
                        name=f"ps{it}_{q}_{s}_{half}", tag="ps",
                        padded_shape=[128, 1024])
            for p in range(2):
                g = 2 * q + p
                for half in range(NW // 2):
                    for wsub in range(2):
                        wt = half * 2 + wsub
                        for s in range(2):
                            base = bases[s]
                            lhs = xt[base:base + C,
                                     g * W + wt * 128: g * W + wt * 128 + 128]
                            lo = LEAD if (it == 0 and g == 0 and s == 0
                                          and wt == 0) else 0
                            rhs = yt[base:base + C,
                                     g * W + wt * 128 + lo:
                                     g * W + wt * 128 + MMN]
                            nc.tensor.matmul(
                                ps[s, half][:, 512 * p + wsub * MMN + lo:
                                            512 * p + (wsub + 1) * MMN],
                                lhs, rhs, start=True, stop=True)

            for s in range(2):
                gt = gts[s]
                for half in range(NW // 2):
                    # half 0 skips u<47 (w-tile 0's w<d zone, memset to 0)
                    lo = LEAD if half == 0 else 0
                    gfull = gt[:, :]
                    dst = bass.AP(
                        gfull.tensor,
                        gfull.offset + (half * 2 * MMN + lo) * HB + 2 * q,
                        [[GW8, 128], [HB, 2 * MMN - lo], [1, 2]])
                    pfull = ps[s, half][:, :]
                    src = bass.AP(
                        pfull.tensor, pfull.offset + lo,
                        [[1024, 128], [1, 2 * MMN - lo], [512, 2]])
                    if (dcount % 13) < act_frac * 13:
                        nc.scalar.mul(dst, src, inv_c)
                    else:
                        nc.vector.tensor_scalar_mul(dst, src, inv_c)
                    dcount += 1

        gath = []
        for s in range(2):
            pc = pcs[s]
            eng = nc.sync if s == 0 else nc.scalar
            gt3d = gts[s][:, :].rearrange("p (w c) -> p w c", c=MMN * HB)
            for half in range(2):        # row-halves j in [0,64), [64,128)
                scr0 = pc * SBLK + half * PBLK
                dmp_src = gt3d[64 * half:64 * half + 64, :,
                               half * 512:half * 512 + PW]
                dmp_dst = bass.AP(gd, scr0, [[PROW, 64], [PW, NW], [1, PW]])
                eng.dma_start(dmp_dst, dmp_src)
                # skewed extraction: 768B diagonal runs -> contiguous output
                g_src = bass.AP(gd, scr0,
                                [[PROW + HB, 64], [PW, NW], [1, DH]])
                g_dst = bass.AP(o_t, pc * (128 * NW * DH)
                                + half * (64 * NW * DH),
                                [[NW * DH, 64], [DH, NW], [1, DH]])
                gath.append((eng, g_dst, g_src))
        for eng, g_dst, g_src in pending:
            eng.dma_start(g_dst, g_src)
        pending = gath

    for eng, g_dst, g_src in pending:
        eng.dma_start(g_dst, g_src)


def _build_kernel():
    nc = bass.Bass(trn_type="TRN2", target_bir_lowering=False)
    x_d = nc.dram_tensor("x", [C, H, W], mybir.dt.float32, kind="ExternalInput")
    y_d = nc.dram_tensor("y", [C, H, W], mybir.dt.float32, kind="ExternalInput")
    o_d = nc.dram_tensor("o", [NBLK, 128, NW, DH], mybir.dt.float16,
                         kind="ExternalOutput")
    with ExitStack() as ctx:
        tc = ctx.enter_context(tile.TileContext(nc))
        _emit_body(ctx, tc, x_d.ap(), y_d.ap(), o_d.ap())
    _split_waits(nc)
    return nc


_NC_CACHE = None


def _get_nc():
    global _NC_CACHE
    if _NC_CACHE is None:
        _NC_CACHE = _build_kernel()
    return _NC_CACHE


def kernel(x: np.ndarray, y: np.ndarray, maxdisp=48) -> np.ndarray:
    assert int(maxdisp) == D
    x = np.ascontiguousarray(np.asarray(x, dtype=np.float32))
    y = np.ascontiguousarray(np.asarray(y, dtype=np.float32))
    assert x.shape == (B, C, H, W) and y.shape == (B, C, H, W)

    nc = _get_nc()
    in_maps = [{"x": x[b], "y": y[b]} for b in range(B)]
    res = bass_utils.run_bass_kernel_spmd(nc, in_maps, core_ids=list(range(B)))

    out = np.empty((B, D, H, W), dtype=np.float32)
    for b in range(B):
        ob = np.asarray(res.results[b]["o"])      # [NBLK, 128, NW, DH] fp16
        arr = ob.reshape(NBLK, 128, NW, D, HB)    # [hblk, j, wt, d_rev, hsub]
        out[b] = (arr[:, :, :, ::-1, :]
                  .transpose(3, 0, 4, 2, 1)       # [d, hblk, hsub, wt, j]
                  .reshape(D, H, W)
                  .astype(np.float32))
    return out


if __name__ == "__main__":
    rng = np.random.default_rng(0)
    x = rng.standard_normal((B, C, H, W), dtype=np.float32)
    y = rng.standard_normal((B, C, H, W), dtype=np.float32)
    out = kernel(x=x, y=y, maxdisp=D)
    print("kernel output:", out.shape, out.dtype)
